# revision 5
# baseline (speedup 1.0000x reference)
"""Trainium2 Bass kernel: 3x3 valid conv (64ch -> 128ch) + per-pixel bias.

Strategy: shard the 510 output rows spatially across 8 NeuronCores (64
rows/core with a 2-row input halo; core 7 overlaps core 6 by 2 rows).
Inside a core, the 64-row band is split across the two PE row-strips:
partitions 0-63 hold the input rows for output rows 0-31 of the band,
partitions 64-127 the rows for output rows 32-63 (the host feeds the
band pre-split so every DMA runs at full 128-partition width).  Each
output row is 9 accumulating K=64 matmuls (one per kernel tap, N=510);
the two strips run concurrently on disjoint PE row-halves, so a
tap-pair costs one N=510 stream (~213ns warm) and the 288 pair-streams
put the PE at its 61us roofline for this shard.

Everything rides HBM as fp16: the PE streams 16-bit operands at the
same 1 col/cycle as fp32r, so halving every tensor's bytes moves the
kernel from DMA-bound (34.6MB at the ~420 GB/s fabric cap = 82us) to
PE-bound (21.5MB = 51us of DMA hidden under the 62us matmul stream).
fp16's 10 mantissa bits keep the end-to-end error ~1e-3 of output
absmax.  fp16 weights also enable the compiler's fast-weight-load path
(32-bit reads), so the per-tap LDWEIGHTS fully hides under the stream.

Ring plan: sync carries w + all bias + strip-a stores; scalar carries
the input chunks (first-matmul critical path) + strip-b stores.  Bias
is fully buffered in SBUF (8 group buffers per strip) so every PSUM
evacuation is a single fused DVE add; the last two groups store row by
row on both rings so the final drain starts as soon as each row lands.
PSUM is accumulated in fp32 and only rounded to fp16 once, at the
bias-add.  Output is converted back to fp32 on the host.
"""

import numpy as np
from contextlib import ExitStack

import concourse.bass as bass
import concourse.tile as tile
from concourse import bacc, mybir
from concourse import bass_utils

C, H, W = 64, 512, 512
D, KK = 128, 3
OH, OW = H - KK + 1, W - KK + 1          # 510, 510
NCORES = 8
RPC = 64                                  # output rows per core
BAND = RPC + KK - 1                       # 66 input rows per core
HALF = RPC // 2                           # 32 output rows per strip
IBAND = HALF + KK - 1                     # 34 input rows per strip
GROUPS = 8
GROWS = HALF // GROUPS                    # 4 pair-rows per group

f32 = mybir.dt.float32
f16 = mybir.dt.float16

# row offset of each core's output band
STARTS = [min(i * RPC, OH - RPC) for i in range(NCORES)]

_CACHE = {}

# results of the last hardware run (inspected by test harnesses)
LAST_RESULTS = None


def _build_program():
    nc = bacc.Bacc(
        "TRN2", target_bir_lowering=False, debug=False, num_devices=NCORES
    )
    # x is pre-split on the host: row (h*64+c) holds band rows
    # [32h, 32h+34) of channel c, flattened
    x = nc.dram_tensor("x", [2 * C, IBAND * W], f16, kind="ExternalInput").ap()
    # w is pre-duplicated: rows 0-63 and 64-127 identical, [c, (ky kx d)]
    w = nc.dram_tensor("w", [2 * C, 9 * D], f16, kind="ExternalInput").ap()
    b = nc.dram_tensor("b", [D, RPC, OW], f16, kind="ExternalInput").ap()
    y = nc.dram_tensor("y", [D, RPC, OW], f16, kind="ExternalOutput").ap()

    b_flat = b.rearrange("d r x -> d (r x)")
    y_flat = y.rearrange("d r x -> d (r x)")

    with tile.TileContext(nc) as tc:
        with ExitStack() as ctx:
            xp = ctx.enter_context(tc.tile_pool(name="xin", bufs=1))
            wp = ctx.enter_context(tc.tile_pool(name="wt", bufs=1))
            bp = ctx.enter_context(tc.tile_pool(name="bias", bufs=8))
            op = ctx.enter_context(tc.tile_pool(name="out", bufs=4))
            pp = ctx.enter_context(tc.tile_pool(name="ps", bufs=3, space="PSUM"))
            wmp = ctx.enter_context(tc.tile_pool(name="warm", bufs=1, space="PSUM"))

            # HAM pre-warm: the PE clock sits at 1.2 GHz until ~3.4us of
            # sustained matmul activity.  Burn that window on dummy
            # matmuls over a memset scratch tile while the input DMA is
            # in flight, so the real stream runs at 2.4 GHz from pair 0.
            sc = wp.tile([128, 128], f16, tag="scratch")
            nc.gpsimd.memset(sc[:], 0.0)
            wps = wmp.tile([128, 512], f32)
            for _ in range(34):
                nc.tensor.matmul(
                    wps[:, 0:128], sc[:], sc[:], start=True, stop=True
                )

            # critical path to the first matmul: weights on sync, first
            # input chunk on scalar, concurrently
            wt = wp.tile([128, 9 * D], f16)
            nc.sync.dma_start(wt[:], w[:, :])

            xin = xp.tile([128, IBAND * W], f16)
            bounds = [0, 2, 5, 10, 16, 24, IBAND]
            for ci in range(len(bounds) - 1):
                r0, r1 = bounds[ci], bounds[ci + 1]
                nc.scalar.dma_start(
                    xin[:, r0 * W:r1 * W], x[:, r0 * W:r1 * W]
                )

            # all bias groups ride the sync ring behind the weights; SBUF
            # holds all 16 group-tiles so every load issues immediately
            bias_tiles = []
            for g in range(GROUPS):
                ra, rb = g * GROWS, HALF + g * GROWS
                ba = bp.tile([128, GROWS * OW], f16, tag="ba")
                nc.sync.dma_start(ba[:], b_flat[:, ra * OW:(ra + GROWS) * OW])
                bb = bp.tile([128, GROWS * OW], f16, tag="bb")
                nc.sync.dma_start(bb[:], b_flat[:, rb * OW:(rb + GROWS) * OW])
                bias_tiles.append((ba, bb))

            for g in range(GROUPS):
                ra = g * GROWS                 # band rows ra..ra+3  (strip 0)
                rb = HALF + ra                 # band rows rb..rb+3  (strip 1)
                ba, bb = bias_tiles[g]
                ya = op.tile([128, GROWS * OW], f16, tag="ya")
                yb = op.tile([128, GROWS * OW], f16, tag="yb")

                last = g == GROUPS - 1
                for j in range(GROWS):
                    yl = ra + j                # strip-local output row
                    pa = pp.tile([128, OW], f32, tag="pa")
                    pb = pp.tile([128, OW], f32, tag="pb")
                    for t in range(9):
                        ky, kx = divmod(t, 3)
                        off = (yl + ky) * W + kx
                        nc.tensor.matmul(
                            pa[:],
                            wt[0:64, t * D:(t + 1) * D],
                            xin[0:64, off:off + OW],
                            start=(t == 0), stop=(t == 8),
                        )
                        nc.tensor.matmul(
                            pb[:],
                            wt[64:128, t * D:(t + 1) * D],
                            xin[64:128, off:off + OW],
                            start=(t == 0), stop=(t == 8),
                        )
                    sl = slice(j * OW, (j + 1) * OW)
                    if last and j == GROWS - 1:
                        # the very last row: column-split the bias-add and
                        # the store so the final HBM write starts one DVE
                        # half-add (not two full adds) after the last
                        # matmul, interleaved across both rings
                        HC = OW // 2
                        for c0, c1 in ((0, HC), (HC, OW)):
                            ch = slice(j * OW + c0, j * OW + c1)
                            nc.vector.tensor_add(
                                yb[:, ch], pb[:, c0:c1], bb[:, ch]
                            )
                            nc.sync.dma_start(
                                y_flat[:, (rb + j) * OW + c0:
                                       (rb + j) * OW + c1],
                                yb[:, ch],
                            )
                            nc.vector.tensor_add(
                                ya[:, ch], pa[:, c0:c1], ba[:, ch]
                            )
                            nc.scalar.dma_start(
                                y_flat[:, (ra + j) * OW + c0:
                                       (ra + j) * OW + c1],
                                ya[:, ch],
                            )
                    else:
                        nc.vector.tensor_add(ya[:, sl], pa[:], ba[:, sl])
                        nc.vector.tensor_add(yb[:, sl], pb[:], bb[:, sl])

                if last:
                    # rows 0..GROWS-2 store per-row (row GROWS-1 was
                    # stored above, split by column halves)
                    for h in range(GROWS - 1):
                        cs = slice(h * OW, (h + 1) * OW)
                        nc.scalar.dma_start(
                            y_flat[:, (ra + h) * OW:(ra + h + 1) * OW],
                            ya[:, cs],
                        )
                        nc.sync.dma_start(
                            y_flat[:, (rb + h) * OW:(rb + h + 1) * OW],
                            yb[:, cs],
                        )
                elif g == GROUPS - 2:
                    # tail: per-row stores, one strip per ring, so the
                    # final drain starts as soon as each row's add lands
                    for h in range(GROWS):
                        cs = slice(h * OW, (h + 1) * OW)
                        nc.scalar.dma_start(
                            y_flat[:, (ra + h) * OW:(ra + h + 1) * OW],
                            ya[:, cs],
                        )
                        nc.sync.dma_start(
                            y_flat[:, (rb + h) * OW:(rb + h + 1) * OW],
                            yb[:, cs],
                        )
                else:
                    nc.sync.dma_start(
                        y_flat[:, ra * OW:(ra + GROWS) * OW], ya[:]
                    )
                    nc.scalar.dma_start(
                        y_flat[:, rb * OW:(rb + GROWS) * OW], yb[:]
                    )

    nc.compile()
    return nc


def kernel(input, kernels, biases):
    global LAST_RESULTS
    if "nc" not in _CACHE:
        _CACHE["nc"] = _build_program()
    nc = _CACHE["nc"]

    xr = np.asarray(input, dtype=np.float16)                   # [C, H, W]
    w1 = np.ascontiguousarray(
        np.asarray(kernels, dtype=np.float32).transpose(1, 2, 3, 0)
    ).reshape(C, 9 * D).astype(np.float16)
    wr = np.concatenate([w1, w1], axis=0)                      # [128, 9*D]
    br = np.asarray(biases, dtype=np.float16)

    in_maps = []
    for s in STARTS:
        band = xr[:, s:s + BAND, :]
        xs = np.concatenate(
            [band[:, 0:IBAND, :], band[:, HALF:HALF + IBAND, :]], axis=0
        ).reshape(2 * C, IBAND * W)
        in_maps.append({
            "x": np.ascontiguousarray(xs),
            "w": wr,
            "b": np.ascontiguousarray(br[:, s:s + RPC, :]),
        })

    res = bass_utils.run_bass_kernel_spmd(
        nc, in_maps, core_ids=list(range(NCORES))
    )
    LAST_RESULTS = res

    out = np.empty((D, OH, OW), np.float32)
    for i, s in enumerate(STARTS):
        out[:, s:s + RPC, :] = res.results[i]["y"].astype(np.float32)
    return out
